# revision 30
# baseline (speedup 1.0000x reference)
"""Bipolar self-attention on 8 Trainium2 NeuronCores.

Sharding: data-parallel over batch (B=2 -> 2 groups of 4 cores), tensor-
parallel over heads within a group (16 heads -> 4 heads/core). Each core:
  - projects its head-slice of Q/K transposed ([c, n] layout) and V natural,
    with the bipolar transform (q-0.5)*2 and the 1/sqrt(Dh) score scale
    folded into the projection weights/biases host-side,
  - computes S^T = Kb Qb^T per head tile-by-tile, exponentiates (softmax
    without max subtraction -- scores are O(10), exp is safe in fp32),
  - multiplies P^T by V augmented with a ones column, so the softmax
    denominator falls out of the same matmul (row 64 of the accumulator),
  - normalizes and applies its slice of the output projection (row-parallel).
Host sums the 4 partial outputs per batch and adds the bias terms.

v5 structure:
  - all matmul operands in bf16 (same PE rate as f32r at moving>=256, half
    the HBM input traffic and SBUF footprint),
  - x host-packed N-CHUNK-major ([128, 4, 8, 512]): the first 1MB chunk
    already carries every contraction slice for n 0-511, so the k/q
    projections (and attention) start as soon as it lands (~12us) instead
    of waiting for all of x,
  - ONE continuous software-pipelined attention stream across all 8
    (window, head-pair) tiles: P*V lags scores by 2 steps globally, so
    the PE and ACT never drain at tile boundaries. All pair-0 (c-tile 0)
    windows run first so the ct1 projections have 48 steps of slack,
  - every projection runs as a rotating single-bank PSUM group scheduled
    as per-step PE filler work with deadlines ahead of its consumers,
  - softmax denominators batch-transposed through DRAM into [128, 8] for
    one cheap exact reciprocal per head-pair; the last pair instead uses
    a direct [2, 512] reciprocal to cut two DMA hops off the kernel tail.
"""

import numpy as np
import ml_dtypes

import concourse.bass as bass
import concourse.tile as tile
from concourse import bacc, mybir
from concourse.bass_utils import run_bass_kernel_spmd

D_MODEL = 1024
NHEAD = 16
HEAD_DIM = 64
B = 2
N = 2048
N_CORES = 8
HEADS_PER_CORE = NHEAD // (N_CORES // B)  # 4
C_LOC = HEADS_PER_CORE * HEAD_DIM  # 256

F32 = mybir.dt.float32
BF16 = mybir.dt.bfloat16

_CACHE = {}

NT = N // 128          # 16 k tiles
DC = D_MODEL // 128    # 8 contraction chunks
CT = C_LOC // 128      # 2 local-channel tiles
QW = 512               # q window width
NW = N // QW           # 4 q windows / n chunks
WCOL = DC * C_LOC      # 2048 columns per packed weight

# global attention stream: all c-tile-0 pairs first, then c-tile-1
PAIR_SEQ = [(0, 0), (1, 0), (2, 0), (3, 0), (0, 1), (1, 1), (2, 1), (3, 1)]


def build_nc():
    nc = bacc.Bacc("TRN2", target_bir_lowering=False, debug=False)

    xP = nc.dram_tensor("xP", [128, NW, DC * QW], BF16, kind="ExternalInput")
    wall = nc.dram_tensor("wall", [128, 4 * WCOL], BF16, kind="ExternalInput")
    ball = nc.dram_tensor("ball", [128, 2 * CT], F32, kind="ExternalInput")
    y = nc.dram_tensor("y", [N, D_MODEL], F32, kind="ExternalOutput")

    with tile.TileContext(nc) as tc:
        with (
            tc.tile_pool(name="singles", bufs=1) as singles,
            tc.tile_pool(name="pt", bufs=3) as ptp,
            tc.tile_pool(name="ovs", bufs=4) as ovsp,
            tc.tile_pool(name="norm", bufs=3) as normp,
            tc.tile_pool(name="yout", bufs=4) as youtp,
        ):
            # sync ring: the four x n-chunks. scalar ring: q/k weights
            # first (wqk and xn0 land together ~12us), then v/o weights,
            # then the tiny strided bias transfer (only needed at the
            # first bias-add, and its 128 16-byte descriptors would stall
            # the ring if issued first).
            # weight pack order is wk|wq|wv|wo: wk ships alone first so the
            # k projection (the stream prologue) starts ~1.4us sooner
            wall_sb = singles.tile([128, 4 * WCOL], BF16)
            nc.scalar.dma_start(wall_sb[:, 0:WCOL], wall.ap()[:, 0:WCOL])
            nc.scalar.dma_start(wall_sb[:, WCOL:2 * WCOL],
                                wall.ap()[:, WCOL:2 * WCOL])
            # x resident as [128, nch, dc, 512]
            xT_sb = singles.tile([128, NW, DC, QW], BF16)
            for nch in range(NW):
                nc.sync.dma_start(
                    xT_sb[:, nch].rearrange("p c j -> p (c j)"),
                    xP.ap()[:, nch],
                )
            nc.scalar.dma_start(wall_sb[:, 2 * WCOL:4 * WCOL],
                                wall.ap()[:, 2 * WCOL:4 * WCOL])
            ball_sb = singles.tile([128, 2 * CT], F32)
            nc.gpsimd.dma_start(ball_sb[:], ball.ap())

            wkT_sb = wall_sb[:, 0 * WCOL:1 * WCOL].rearrange(
                "p (c m) -> p c m", c=DC)
            wqT_sb = wall_sb[:, 1 * WCOL:2 * WCOL].rearrange(
                "p (c m) -> p c m", c=DC)
            wvT_sb = wall_sb[:, 2 * WCOL:3 * WCOL].rearrange(
                "p (c m) -> p c m", c=DC)
            woT_sb = wall_sb[:, 3 * WCOL:4 * WCOL].rearrange(
                "p (c m) -> p c m", c=CT)
            bq_sb = ball_sb[:, 0:CT]
            bk_sb = ball_sb[:, CT:2 * CT]

            qT_sb = singles.tile([128, CT, N], BF16)
            kT_sb = singles.tile([128, CT, N], BF16)
            v1_sb = singles.tile([128, NT, HEADS_PER_CORE, HEAD_DIM + 1], BF16)
            ones_sb = singles.tile([128, NT * HEADS_PER_CORE], F32)
            nc.vector.memset(ones_sb[:], 1.0)
            nc.vector.tensor_copy(
                v1_sb[:, :, :, HEAD_DIM],
                ones_sb[:].rearrange("p (n h) -> p n h", h=HEADS_PER_CORE),
            )
            outT_sb = singles.tile([128, CT, N], BF16)

            with (
                tc.tile_pool(name="pp", bufs=1, space="PSUM") as pps,
                tc.tile_pool(name="pp2", bufs=1, space="PSUM") as pp2s,
                tc.tile_pool(name="st", bufs=2, space="PSUM") as stp,
                tc.tile_pool(name="ov", bufs=2, space="PSUM") as ovp,
                tc.tile_pool(name="dsc", bufs=4, space="DRAM") as dscp,
            ):
                # ---- PE warm-up: the tensor engine needs ~3us of
                # continuous work to reach its 2.4 GHz p-state; burn the
                # otherwise-idle DMA wait on dummy matmuls so the real
                # prologue starts at full clock.
                wsa = singles.tile([128, HEAD_DIM], BF16, name="wsa")
                nc.vector.memset(wsa[:], 1.0)
                wsb = singles.tile([128, QW], BF16, name="wsb")
                nc.vector.memset(wsb[:], 1.0)
                for _ in range(16):
                    wps = stp.tile([128, 2 * QW], F32, tag="st", name="wps")
                    nc.tensor.matmul(wps[0:HEAD_DIM, 0:QW], wsa[:], wsb[:],
                                     start=True, stop=True)

                def v_proj_tile(nt):
                    ps = pps.tile([128, QW], F32, tag="pp", name="vps")
                    nch, sub = nt // 4, nt % 4
                    for dc in range(DC):
                        nc.tensor.matmul(
                            ps[:, :C_LOC],
                            xT_sb[:, nch, dc, sub * 128:(sub + 1) * 128],
                            wvT_sb[:, dc, :],
                            start=(dc == 0),
                            stop=(dc == DC - 1),
                        )
                    nc.vector.tensor_copy(
                        v1_sb[:, nt, :, 0:HEAD_DIM],
                        ps[:, :C_LOC].rearrange("p (h d) -> p h d",
                                                h=HEADS_PER_CORE),
                    )

                # rotating single-bank projection group: 8 single-matmul
                # thunks accumulating one [128, 512] n-chunk of q or k
                def proj_group(w_sb, b_sb, dst, ct, nch, pool, tag):
                    state = {}

                    def mm(dc):
                        if dc == 0:
                            state["ps"] = pool.tile(
                                [128, QW], F32, tag=tag, name=tag + "ps")
                        nc.tensor.matmul(
                            state["ps"][:],
                            w_sb[:, dc, ct * 128:(ct + 1) * 128],
                            xT_sb[:, nch, dc, :],
                            start=(dc == 0),
                            stop=(dc == DC - 1),
                        )
                        if dc == DC - 1:
                            nc.vector.tensor_tensor(
                                dst[:, ct, nch * QW:(nch + 1) * QW],
                                state["ps"][:],
                                b_sb[:, ct:ct + 1].to_broadcast((128, QW)),
                                mybir.AluOpType.add,
                            )

                    return [lambda dc=dc: mm(dc) for dc in range(DC)]

                def y_unit(qq, nt_i, cok, ps_pool=None, ps_tag="pp",
                           ring=None):
                    nt = qq * (QW // 128) + nt_i
                    ps = (ps_pool or pps).tile(
                        [128, QW], F32, tag=ps_tag, name="yps")
                    for ct in range(CT):
                        nc.tensor.matmul(
                            ps[:],
                            outT_sb[:, ct, nt * 128:(nt + 1) * 128],
                            woT_sb[:, ct, cok * QW:(cok + 1) * QW],
                            start=(ct == 0),
                            stop=(ct == CT - 1),
                        )
                    ys = youtp.tile([128, QW], F32, tag="ys", name="ys")
                    nc.vector.tensor_copy(ys[:], ps[:])
                    (ring or nc.gpsimd).dma_start(
                        y.ap()[nt * 128:(nt + 1) * 128,
                               cok * QW:(cok + 1) * QW],
                        ys[:],
                    )

                def y_units(qq):
                    # alternate the PSUM accumulator between the pp and the
                    # (post-projection idle) pp2 bank so consecutive units
                    # don't serialize on one slot's copy-out
                    slots = [(pps, "pp"), (pp2s, "pp2")]
                    out = []
                    for i, (nt_i, cok) in enumerate(
                            (a, b) for a in range(QW // 128)
                            for b in range(D_MODEL // QW)):
                        pool, tag = slots[i % 2]
                        out.append(lambda nt_i=nt_i, cok=cok, pool=pool,
                                   tag=tag: y_unit(qq, nt_i, cok,
                                                   ps_pool=pool, ps_tag=tag))
                    return out

                def normalize_tail(qq, pair, ovA, ovB):
                    # last pair: everything on-chip. Direct DVE reciprocal
                    # on the denominator row (3.3us, but no DMA latency)
                    # and the partition-broadcast via a rank-1 PE matmul
                    # into PSUM. Zero DMA hops on the kernel's tail.
                    q0 = qq * QW
                    ovsA = ovsp.tile([HEAD_DIM + 1, QW], F32, tag="ovs",
                                     name="ovsA")
                    nc.vector.tensor_copy(ovsA[:], ovA[:])
                    ovsB = ovsp.tile([HEAD_DIM + 1, QW], F32, tag="ovs",
                                     name="ovsB")
                    nc.vector.tensor_copy(ovsB[:], ovB[:])
                    for half, ovs in ((0, ovsA), (1, ovsB)):
                        rec = normp.tile([1, QW], F32, tag="rec", name="rec")
                        nc.vector.reciprocal(
                            rec[:], ovs[HEAD_DIM:HEAD_DIM + 1, :])
                        bcp = ovp.tile([64, QW], F32, tag="ov", name="bcp")
                        nc.tensor.matmul(bcp[:], ones_sb[0:1, 0:HEAD_DIM],
                                         rec[:], start=True, stop=True)
                        nc.vector.tensor_mul(
                            outT_sb[64 * half:64 * half + 64, pair,
                                    q0:q0 + QW],
                            ovs[0:HEAD_DIM, :],
                            bcp[:],
                        )

                def normalize(qq, pair, ovA, ovB):
                    q0 = qq * QW
                    # denominators batch-transposed through DRAM into
                    # [128, 8] for one cheap exact reciprocal
                    ovsA = ovsp.tile([HEAD_DIM + 1, QW], F32, tag="ovs",
                                     name="ovsA")
                    nc.vector.tensor_copy(ovsA[:], ovA[:])
                    ovsB = ovsp.tile([HEAD_DIM + 1, QW], F32, tag="ovs",
                                     name="ovsB")
                    nc.vector.tensor_copy(ovsB[:], ovB[:])
                    dn = dscp.tile([2, QW], F32, name="dn")
                    nc.sync.dma_start(dn[0:1], ovsA[HEAD_DIM:HEAD_DIM + 1, :])
                    nc.sync.dma_start(dn[1:2], ovsB[HEAD_DIM:HEAD_DIM + 1, :])
                    dn_t = dn[:].rearrange("a q -> (a q)").rearrange(
                        "(p c) -> p c", p=128)
                    g = normp.tile([128, 2 * QW // 128], F32, tag="g",
                                   name="g")
                    nc.sync.dma_start(g[:], dn_t)
                    g2 = normp.tile([128, 2 * QW // 128], F32, tag="g2",
                                    name="g2")
                    nc.vector.reciprocal(g2[:], g[:])
                    rd = dscp.tile([2, QW], F32, name="rd")
                    rd_t = rd[:].rearrange("a q -> (a q)").rearrange(
                        "(p c) -> p c", p=128)
                    nc.sync.dma_start(rd_t, g2[:])
                    for half, ovs in ((0, ovsA), (1, ovsB)):
                        bc = normp.tile([64, QW], F32, tag="bc", name="bc")
                        nc.gpsimd.dma_start(
                            bc[:], rd[half:half + 1].partition_broadcast(64)
                        )
                        nc.vector.tensor_mul(
                            outT_sb[64 * half:64 * half + 64, pair,
                                    q0:q0 + QW],
                            ovs[0:HEAD_DIM, :],
                            bc[:],
                        )

                def spread(units, s0, s1):
                    sched = {}
                    nst = s1 - s0 + 1
                    done = 0
                    for i in range(nst):
                        want = (i + 1) * len(units) // nst
                        if want > done:
                            sched.setdefault(s0 + i, []).extend(
                                units[done:want])
                            done = want
                    return sched

                def merge(*scheds):
                    out = {}
                    for s in scheds:
                        for k, v in s.items():
                            out.setdefault(k, []).extend(v)
                    return out

                # ---- prologue: k/q n-chunk 0 c-tile 0 (dc-inner; only
                # needs the first x chunk) so the stream can start at once
                kn0 = proj_group(wkT_sb, bk_sb, kT_sb, 0, 0, pp2s, "pp2")
                qn0 = proj_group(wqT_sb, bq_sb, qT_sb, 0, 0, pps, "pp")
                for f in kn0:
                    f()
                for f in qn0:
                    f()

                # ---- filler schedule over the 128-step global stream
                vg = [lambda nt=nt: v_proj_tile(nt) for nt in range(NT)]
                pg = lambda w, b, dst, ct, nch: proj_group(
                    w, b, dst, ct, nch, pp2s, "pp2")
                kn1 = pg(wkT_sb, bk_sb, kT_sb, 0, 1)
                kn2 = pg(wkT_sb, bk_sb, kT_sb, 0, 2)
                kn3 = pg(wkT_sb, bk_sb, kT_sb, 0, 3)
                qn1 = pg(wqT_sb, bq_sb, qT_sb, 0, 1)
                qn2 = pg(wqT_sb, bq_sb, qT_sb, 0, 2)
                qn3 = pg(wqT_sb, bq_sb, qT_sb, 0, 3)
                k1 = [pg(wkT_sb, bk_sb, kT_sb, 1, n) for n in range(4)]
                q1 = [pg(wqT_sb, bq_sb, qT_sb, 1, n) for n in range(4)]
                sched = merge(
                    spread(vg, 0, 13),        # OV(j) at step j+2
                    spread(kn1, 0, 2),        # S(4) reads kT ct0 chunk 1
                    spread(kn2, 3, 6),        # S(8) chunk 2
                    spread(kn3, 7, 10),       # S(12) chunk 3
                    spread(qn1, 11, 14),      # S(16) reads qT ct0 win 1
                    spread(qn2, 16, 23),      # S(32) win 2
                    spread(qn3, 32, 39),      # S(48) win 3
                    spread(q1[0], 40, 47),    # S(64) reads qT ct1 win 0
                    spread(k1[0], 48, 53),    # S(64) reads kT ct1 chunk 0
                    spread(k1[1], 54, 59),    # S(68) chunk 1
                    spread(k1[2], 60, 65),    # S(72) chunk 2
                    spread(k1[3], 66, 73),    # S(76) chunk 3
                    spread(q1[1], 74, 79),    # S(80) qT ct1 win 1
                    spread(q1[2], 80, 87),    # S(96) win 2
                    spread(q1[3], 88, 95),    # S(112) win 3
                    spread(y_units(0), 96, 107),
                    spread(y_units(1), 108, 117),
                    spread(y_units(2), 118, 127),
                )

                # ---- the continuous attention stream
                pstate = {}

                def S(pi, kt):
                    qq, pair = PAIR_SEQ[pi]
                    q0 = qq * QW
                    st = stp.tile([128, 2 * QW], F32, tag="st", name="st")
                    for half, p0 in ((0, 0), (1, 64)):
                        nc.tensor.matmul(
                            st[:, half * QW:(half + 1) * QW],
                            kT_sb[p0:p0 + 64, pair, kt * 128:(kt + 1) * 128],
                            qT_sb[p0:p0 + 64, pair, q0:q0 + QW],
                            start=True,
                            stop=True,
                        )
                    pt = ptp.tile([128, 2 * QW], BF16, tag="pt", name="pt")
                    nc.scalar.activation(
                        pt[:], st[:], mybir.ActivationFunctionType.Exp
                    )
                    pstate[pi]["pts"][kt] = pt

                def OV(pi, kt):
                    qq, pair = PAIR_SEQ[pi]
                    ps = pstate[pi]
                    if kt == 0:
                        ps["ovA"] = ovp.tile([HEAD_DIM + 1, QW], F32,
                                             tag="ov", name="ovA")
                        ps["ovB"] = ovp.tile([HEAD_DIM + 1, QW], F32,
                                             tag="ov", name="ovB")
                    pt = ps["pts"].pop(kt)
                    for half, ov in ((0, ps["ovA"]), (1, ps["ovB"])):
                        nc.tensor.matmul(
                            ov[:],
                            v1_sb[:, kt, 2 * pair + half, :],
                            pt[:, half * QW:(half + 1) * QW],
                            start=(kt == 0),
                            stop=(kt == NT - 1),
                        )
                    if kt == NT - 1:
                        if pi == len(PAIR_SEQ) - 1:
                            normalize_tail(qq, pair, ps["ovA"], ps["ovB"])
                        else:
                            normalize(qq, pair, ps["ovA"], ps["ovB"])

                NSTEP = len(PAIR_SEQ) * NT
                for step in range(NSTEP + 2):
                    if step < NSTEP:
                        pi, kt = step // NT, step % NT
                        if kt == 0:
                            pstate[pi] = {"pts": {}}
                        S(pi, kt)
                    if step >= 2:
                        OV((step - 2) // NT, (step - 2) % NT)
                    for f in sched.get(step, ()):
                        f()

                # tail: last window's out-proj, PSUM tiles rotated through
                # the now-idle st/pp2 slots, writes split across both rings
                tail_slots = [(stp, "st"), (pp2s, "pp2"), (pps, "pp"),
                              (stp, "st")]
                i = 0
                for nt_i in range(QW // 128):
                    for cok in range(D_MODEL // QW):
                        pool, tag = tail_slots[i % len(tail_slots)]
                        # all tail writes on the sync HWDGE ring: the
                        # gpsimd/SWDGE queue has a ~4.5us Q7 drain that
                        # would land after the final transfer
                        y_unit(3, nt_i, cok, ps_pool=pool, ps_tag=tag,
                               ring=nc.sync)
                        i += 1

    nc.compile()
    return nc


def kernel(x, Wq, bq, Wk, bk, Wv, bv, Wo, bo):
    x = np.asarray(x, dtype=np.float32)
    Wq = np.asarray(Wq, dtype=np.float32)
    Wk = np.asarray(Wk, dtype=np.float32)
    Wv = np.asarray(Wv, dtype=np.float32)
    Wo = np.asarray(Wo, dtype=np.float32)
    bq = np.asarray(bq, dtype=np.float32)
    bk = np.asarray(bk, dtype=np.float32)
    bv = np.asarray(bv, dtype=np.float32)
    bo = np.asarray(bo, dtype=np.float32)

    if "nc" not in _CACHE:
        _CACHE["nc"] = build_nc()
    nc = _CACHE["nc"]

    bf16 = ml_dtypes.bfloat16

    def pack_w(w):  # [D, M] -> [128, (D/128)*M] partition-major
        d, m = w.shape
        return np.ascontiguousarray(
            w.reshape(d // 128, 128, m).transpose(1, 0, 2).reshape(128, -1))

    s = 2.0 / np.sqrt(8.0)  # fold bipolar *2 and score scale (1/8 split per side)
    in_maps = []
    for core in range(N_CORES):
        b = core // (N_CORES // B)
        g = core % (N_CORES // B)
        ch = slice(g * C_LOC, (g + 1) * C_LOC)
        wk_p = pack_w(np.ascontiguousarray((s * Wk[ch, :]).T))
        wq_p = pack_w(np.ascontiguousarray((s * Wq[ch, :]).T))
        wv_p = pack_w(np.ascontiguousarray(Wv[ch, :].T))
        wo_p = pack_w(np.ascontiguousarray(Wo[:, ch].T))
        bq_f = (2.0 * bq[ch] - 1.0) / np.sqrt(8.0)
        bk_f = (2.0 * bk[ch] - 1.0) / np.sqrt(8.0)
        # x n-chunk-major: [128, nch, dc, 512]
        xt = np.ascontiguousarray(x[b].T)  # [D, N]
        x_p = np.ascontiguousarray(
            xt.reshape(DC, 128, NW, QW).transpose(1, 2, 0, 3)
        ).reshape(128, NW, DC * QW)
        in_maps.append({
            "xP": x_p.astype(bf16),
            "wall": np.concatenate([wk_p, wq_p, wv_p, wo_p],
                                   axis=1).astype(bf16),
            "ball": np.concatenate(
                [bq_f.reshape(CT, 128).T, bk_f.reshape(CT, 128).T],
                axis=1).astype(np.float32),
        })

    _CACHE["in_maps"] = in_maps
    res = run_bass_kernel_spmd(nc, in_maps, core_ids=list(range(N_CORES)))

    g_per_b = N_CORES // B
    const = (Wo @ bv + bo).astype(np.float32)  # bv folded through out-proj
    out = np.empty((B, N, D_MODEL), dtype=np.float32)
    for b in range(B):
        acc = res.results[b * g_per_b]["y"].astype(np.float32).copy()
        for g in range(1, g_per_b):
            acc += res.results[b * g_per_b + g]["y"]
        out[b] = acc + const
    return out


# revision 32
# speedup vs baseline: 1.1773x; 1.1773x over previous
"""Bipolar self-attention on 8 Trainium2 NeuronCores.

Sharding: data-parallel over batch (B=2 -> 2 groups of 4 cores), tensor-
parallel over heads within a group (16 heads -> 4 heads/core). Each core:
  - projects its head-slice of Q/K transposed ([c, n] layout) and V natural,
    with the bipolar transform (q-0.5)*2 and the 1/sqrt(Dh) score scale
    folded into the projection weights/biases host-side,
  - computes S^T = Kb Qb^T per head tile-by-tile, exponentiates (softmax
    without max subtraction -- scores are O(10), exp is safe in fp32),
  - multiplies P^T by V augmented with a ones column, so the softmax
    denominator falls out of the same matmul (row 64 of the accumulator),
  - normalizes and applies its slice of the output projection (row-parallel).
Host sums the 4 partial outputs per batch and adds the bias terms.

v5 structure:
  - all matmul operands in bf16 (same PE rate as f32r at moving>=256, half
    the HBM input traffic and SBUF footprint),
  - x host-packed N-CHUNK-major ([128, 4, 8, 512]): the first 1MB chunk
    already carries every contraction slice for n 0-511, so the k/q
    projections (and attention) start as soon as it lands (~12us) instead
    of waiting for all of x,
  - ONE continuous software-pipelined attention stream across all 8
    (window, head-pair) tiles: P*V lags scores by 2 steps globally, so
    the PE and ACT never drain at tile boundaries. All pair-0 (c-tile 0)
    windows run first so the ct1 projections have 48 steps of slack,
  - every projection runs as a rotating single-bank PSUM group scheduled
    as per-step PE filler work with deadlines ahead of its consumers,
  - softmax denominators batch-transposed through DRAM into [128, 8] for
    one cheap exact reciprocal per head-pair; the last pair instead uses
    a direct [2, 512] reciprocal to cut two DMA hops off the kernel tail.
"""

import numpy as np
import ml_dtypes

import concourse.bass as bass
import concourse.tile as tile
from concourse import bacc, mybir
from concourse.bass_utils import run_bass_kernel_spmd

D_MODEL = 1024
NHEAD = 16
HEAD_DIM = 64
B = 2
N = 2048
N_CORES = 8
HEADS_PER_CORE = NHEAD // (N_CORES // B)  # 4
C_LOC = HEADS_PER_CORE * HEAD_DIM  # 256

F32 = mybir.dt.float32
BF16 = mybir.dt.bfloat16

_CACHE = {}

NT = N // 128          # 16 k tiles
DC = D_MODEL // 128    # 8 contraction chunks
CT = C_LOC // 128      # 2 local-channel tiles
QW = 512               # q window width
NW = N // QW           # 4 q windows / n chunks
WCOL = DC * C_LOC      # 2048 columns per packed weight

# global attention stream: all c-tile-0 pairs first, then c-tile-1
PAIR_SEQ = [(0, 0), (1, 0), (2, 0), (3, 0), (0, 1), (1, 1), (2, 1), (3, 1)]


def build_nc():
    nc = bacc.Bacc("TRN2", target_bir_lowering=False, debug=False)

    xP = nc.dram_tensor("xP", [128, NW, DC * QW], BF16, kind="ExternalInput")
    wall = nc.dram_tensor("wall", [128, 4 * WCOL], BF16, kind="ExternalInput")
    ball = nc.dram_tensor("ball", [128, 2 * CT], F32, kind="ExternalInput")
    y = nc.dram_tensor("y", [N, D_MODEL], F32, kind="ExternalOutput")

    with tile.TileContext(nc) as tc:
        with (
            tc.tile_pool(name="singles", bufs=1) as singles,
            tc.tile_pool(name="pt", bufs=4) as ptp,
            tc.tile_pool(name="ovs", bufs=4) as ovsp,
            tc.tile_pool(name="norm", bufs=3) as normp,
            tc.tile_pool(name="yout", bufs=4) as youtp,
        ):
            # sync ring: the four x n-chunks. scalar ring: q/k weights
            # first (wqk and xn0 land together ~12us), then v/o weights,
            # then the tiny strided bias transfer (only needed at the
            # first bias-add, and its 128 16-byte descriptors would stall
            # the ring if issued first).
            # weight pack order is wk|wq|wv|wo: wk ships alone first so the
            # k projection (the stream prologue) starts ~1.4us sooner
            wall_sb = singles.tile([128, 4 * WCOL], BF16)
            nc.scalar.dma_start(wall_sb[:, 0:WCOL], wall.ap()[:, 0:WCOL])
            nc.scalar.dma_start(wall_sb[:, WCOL:2 * WCOL],
                                wall.ap()[:, WCOL:2 * WCOL])
            # x resident as [128, nch, dc, 512]
            xT_sb = singles.tile([128, NW, DC, QW], BF16)
            for nch in range(NW):
                nc.sync.dma_start(
                    xT_sb[:, nch].rearrange("p c j -> p (c j)"),
                    xP.ap()[:, nch],
                )
            nc.scalar.dma_start(wall_sb[:, 2 * WCOL:4 * WCOL],
                                wall.ap()[:, 2 * WCOL:4 * WCOL])
            ball_sb = singles.tile([128, 2 * CT], F32)
            nc.gpsimd.dma_start(ball_sb[:], ball.ap())

            wkT_sb = wall_sb[:, 0 * WCOL:1 * WCOL].rearrange(
                "p (c m) -> p c m", c=DC)
            wqT_sb = wall_sb[:, 1 * WCOL:2 * WCOL].rearrange(
                "p (c m) -> p c m", c=DC)
            wvT_sb = wall_sb[:, 2 * WCOL:3 * WCOL].rearrange(
                "p (c m) -> p c m", c=DC)
            woT_sb = wall_sb[:, 3 * WCOL:4 * WCOL].rearrange(
                "p (c m) -> p c m", c=CT)
            bq_sb = ball_sb[:, 0:CT]
            bk_sb = ball_sb[:, CT:2 * CT]

            qT_sb = singles.tile([128, CT, N], BF16)
            kT_sb = singles.tile([128, CT, N], BF16)
            v1_sb = singles.tile([128, NT, HEADS_PER_CORE, HEAD_DIM + 1], BF16)
            ones_sb = singles.tile([128, NT * HEADS_PER_CORE], F32)
            nc.vector.memset(ones_sb[:], 1.0)
            nc.vector.tensor_copy(
                v1_sb[:, :, :, HEAD_DIM],
                ones_sb[:].rearrange("p (n h) -> p n h", h=HEADS_PER_CORE),
            )
            outT_sb = singles.tile([128, CT, N], BF16)

            with (
                tc.tile_pool(name="pp", bufs=1, space="PSUM") as pps,
                tc.tile_pool(name="pp2", bufs=1, space="PSUM") as pp2s,
                tc.tile_pool(name="st", bufs=2, space="PSUM") as stp,
                tc.tile_pool(name="ov", bufs=2, space="PSUM") as ovp,
                tc.tile_pool(name="dsc", bufs=4, space="DRAM") as dscp,
            ):
                # ---- PE warm-up: the tensor engine needs ~3us of
                # continuous work to reach its 2.4 GHz p-state; burn the
                # otherwise-idle DMA wait on dummy matmuls so the real
                # prologue starts at full clock.
                wsa = singles.tile([128, HEAD_DIM], BF16, name="wsa")
                nc.vector.memset(wsa[:], 1.0)
                wsb = singles.tile([128, QW], BF16, name="wsb")
                nc.vector.memset(wsb[:], 1.0)
                for _ in range(16):
                    wps = stp.tile([128, 2 * QW], F32, tag="st", name="wps")
                    nc.tensor.matmul(wps[0:HEAD_DIM, 0:QW], wsa[:], wsb[:],
                                     start=True, stop=True)

                def v_proj_tile(nt):
                    ps = pps.tile([128, QW], F32, tag="pp", name="vps")
                    nch, sub = nt // 4, nt % 4
                    for dc in range(DC):
                        nc.tensor.matmul(
                            ps[:, :C_LOC],
                            xT_sb[:, nch, dc, sub * 128:(sub + 1) * 128],
                            wvT_sb[:, dc, :],
                            start=(dc == 0),
                            stop=(dc == DC - 1),
                        )
                    nc.vector.tensor_copy(
                        v1_sb[:, nt, :, 0:HEAD_DIM],
                        ps[:, :C_LOC].rearrange("p (h d) -> p h d",
                                                h=HEADS_PER_CORE),
                    )

                # rotating single-bank projection group: 8 single-matmul
                # thunks accumulating one [128, 512] n-chunk of q or k
                def proj_group(w_sb, b_sb, dst, ct, nch, pool, tag):
                    state = {}

                    def mm(dc):
                        if dc == 0:
                            state["ps"] = pool.tile(
                                [128, QW], F32, tag=tag, name=tag + "ps")
                        nc.tensor.matmul(
                            state["ps"][:],
                            w_sb[:, dc, ct * 128:(ct + 1) * 128],
                            xT_sb[:, nch, dc, :],
                            start=(dc == 0),
                            stop=(dc == DC - 1),
                        )
                        if dc == DC - 1:
                            nc.vector.tensor_tensor(
                                dst[:, ct, nch * QW:(nch + 1) * QW],
                                state["ps"][:],
                                b_sb[:, ct:ct + 1].to_broadcast((128, QW)),
                                mybir.AluOpType.add,
                            )

                    return [lambda dc=dc: mm(dc) for dc in range(DC)]

                def y_unit(qq, nt_i, cok, ps_pool=None, ps_tag="pp",
                           ring=None):
                    nt = qq * (QW // 128) + nt_i
                    ps = (ps_pool or pps).tile(
                        [128, QW], F32, tag=ps_tag, name="yps")
                    for ct in range(CT):
                        nc.tensor.matmul(
                            ps[:],
                            outT_sb[:, ct, nt * 128:(nt + 1) * 128],
                            woT_sb[:, ct, cok * QW:(cok + 1) * QW],
                            start=(ct == 0),
                            stop=(ct == CT - 1),
                        )
                    ys = youtp.tile([128, QW], F32, tag="ys", name="ys")
                    nc.vector.tensor_copy(ys[:], ps[:])
                    (ring or nc.gpsimd).dma_start(
                        y.ap()[nt * 128:(nt + 1) * 128,
                               cok * QW:(cok + 1) * QW],
                        ys[:],
                    )

                def y_units(qq):
                    # alternate the PSUM accumulator between the pp and the
                    # (post-projection idle) pp2 bank so consecutive units
                    # don't serialize on one slot's copy-out
                    slots = [(pps, "pp"), (pp2s, "pp2")]
                    out = []
                    for i, (nt_i, cok) in enumerate(
                            (a, b) for a in range(QW // 128)
                            for b in range(D_MODEL // QW)):
                        pool, tag = slots[i % 2]
                        out.append(lambda nt_i=nt_i, cok=cok, pool=pool,
                                   tag=tag: y_unit(qq, nt_i, cok,
                                                   ps_pool=pool, ps_tag=tag))
                    return out

                def normalize_tail(qq, pair, ovA, ovB):
                    # last pair: everything on-chip. Direct DVE reciprocal
                    # on the denominator row (3.3us, but no DMA latency)
                    # and the partition-broadcast via a rank-1 PE matmul
                    # into PSUM. Zero DMA hops on the kernel's tail.
                    q0 = qq * QW
                    ovsA = ovsp.tile([HEAD_DIM + 1, QW], F32, tag="ovs",
                                     name="ovsA")
                    nc.vector.tensor_copy(ovsA[:], ovA[:])
                    ovsB = ovsp.tile([HEAD_DIM + 1, QW], F32, tag="ovs",
                                     name="ovsB")
                    nc.vector.tensor_copy(ovsB[:], ovB[:])
                    for half, ovs in ((0, ovsA), (1, ovsB)):
                        rec = normp.tile([1, QW], F32, tag="rec", name="rec")
                        nc.vector.reciprocal(
                            rec[:], ovs[HEAD_DIM:HEAD_DIM + 1, :])
                        bcp = ovp.tile([64, QW], F32, tag="ov", name="bcp")
                        nc.tensor.matmul(bcp[:], ones_sb[0:1, 0:HEAD_DIM],
                                         rec[:], start=True, stop=True)
                        nc.vector.tensor_mul(
                            outT_sb[64 * half:64 * half + 64, pair,
                                    q0:q0 + QW],
                            ovs[0:HEAD_DIM, :],
                            bcp[:],
                        )

                def normalize(qq, pair, ovA, ovB):
                    q0 = qq * QW
                    # denominators batch-transposed through DRAM into
                    # [128, 8] for one cheap exact reciprocal
                    ovsA = ovsp.tile([HEAD_DIM + 1, QW], F32, tag="ovs",
                                     name="ovsA")
                    nc.vector.tensor_copy(ovsA[:], ovA[:])
                    ovsB = ovsp.tile([HEAD_DIM + 1, QW], F32, tag="ovs",
                                     name="ovsB")
                    nc.vector.tensor_copy(ovsB[:], ovB[:])
                    dn = dscp.tile([2, QW], F32, name="dn")
                    nc.sync.dma_start(dn[0:1], ovsA[HEAD_DIM:HEAD_DIM + 1, :])
                    nc.sync.dma_start(dn[1:2], ovsB[HEAD_DIM:HEAD_DIM + 1, :])
                    dn_t = dn[:].rearrange("a q -> (a q)").rearrange(
                        "(p c) -> p c", p=128)
                    g = normp.tile([128, 2 * QW // 128], F32, tag="g",
                                   name="g")
                    nc.sync.dma_start(g[:], dn_t)
                    g2 = normp.tile([128, 2 * QW // 128], F32, tag="g2",
                                    name="g2")
                    nc.vector.reciprocal(g2[:], g[:])
                    rd = dscp.tile([2, QW], F32, name="rd")
                    rd_t = rd[:].rearrange("a q -> (a q)").rearrange(
                        "(p c) -> p c", p=128)
                    nc.sync.dma_start(rd_t, g2[:])
                    for half, ovs in ((0, ovsA), (1, ovsB)):
                        bc = normp.tile([64, QW], F32, tag="bc", name="bc")
                        nc.gpsimd.dma_start(
                            bc[:], rd[half:half + 1].partition_broadcast(64)
                        )
                        nc.vector.tensor_mul(
                            outT_sb[64 * half:64 * half + 64, pair,
                                    q0:q0 + QW],
                            ovs[0:HEAD_DIM, :],
                            bc[:],
                        )

                def spread(units, s0, s1):
                    sched = {}
                    nst = s1 - s0 + 1
                    done = 0
                    for i in range(nst):
                        want = (i + 1) * len(units) // nst
                        if want > done:
                            sched.setdefault(s0 + i, []).extend(
                                units[done:want])
                            done = want
                    return sched

                def merge(*scheds):
                    out = {}
                    for s in scheds:
                        for k, v in s.items():
                            out.setdefault(k, []).extend(v)
                    return out

                # ---- prologue: k/q n-chunk 0 c-tile 0 (dc-inner; only
                # needs the first x chunk) so the stream can start at once
                kn0 = proj_group(wkT_sb, bk_sb, kT_sb, 0, 0, pp2s, "pp2")
                qn0 = proj_group(wqT_sb, bq_sb, qT_sb, 0, 0, pps, "pp")
                for f in kn0:
                    f()
                for f in qn0:
                    f()

                # ---- filler schedule over the 128-step global stream
                vg = [lambda nt=nt: v_proj_tile(nt) for nt in range(NT)]
                pg = lambda w, b, dst, ct, nch: proj_group(
                    w, b, dst, ct, nch, pp2s, "pp2")
                kn1 = pg(wkT_sb, bk_sb, kT_sb, 0, 1)
                kn2 = pg(wkT_sb, bk_sb, kT_sb, 0, 2)
                kn3 = pg(wkT_sb, bk_sb, kT_sb, 0, 3)
                qn1 = pg(wqT_sb, bq_sb, qT_sb, 0, 1)
                qn2 = pg(wqT_sb, bq_sb, qT_sb, 0, 2)
                qn3 = pg(wqT_sb, bq_sb, qT_sb, 0, 3)
                k1 = [pg(wkT_sb, bk_sb, kT_sb, 1, n) for n in range(4)]
                q1 = [pg(wqT_sb, bq_sb, qT_sb, 1, n) for n in range(4)]
                sched = merge(
                    spread(vg, 0, 13),        # OV(j) at step j+2
                    spread(kn1, 0, 2),        # S(4) reads kT ct0 chunk 1
                    spread(kn2, 3, 6),        # S(8) chunk 2
                    spread(kn3, 7, 10),       # S(12) chunk 3
                    spread(qn1, 11, 14),      # S(16) reads qT ct0 win 1
                    spread(qn2, 16, 23),      # S(32) win 2
                    spread(qn3, 32, 39),      # S(48) win 3
                    spread(q1[0], 40, 47),    # S(64) reads qT ct1 win 0
                    spread(k1[0], 48, 53),    # S(64) reads kT ct1 chunk 0
                    spread(k1[1], 54, 59),    # S(68) chunk 1
                    spread(k1[2], 60, 65),    # S(72) chunk 2
                    spread(k1[3], 66, 73),    # S(76) chunk 3
                    spread(q1[1], 74, 79),    # S(80) qT ct1 win 1
                    spread(q1[2], 80, 87),    # S(96) win 2
                    spread(q1[3], 88, 95),    # S(112) win 3
                    spread(y_units(0), 96, 107),
                    spread(y_units(1), 108, 117),
                    spread(y_units(2), 118, 127),
                )

                # ---- the continuous attention stream
                pstate = {}

                def S(pi, kt):
                    qq, pair = PAIR_SEQ[pi]
                    q0 = qq * QW
                    st = stp.tile([128, 2 * QW], F32, tag="st", name="st")
                    for half, p0 in ((0, 0), (1, 64)):
                        nc.tensor.matmul(
                            st[:, half * QW:(half + 1) * QW],
                            kT_sb[p0:p0 + 64, pair, kt * 128:(kt + 1) * 128],
                            qT_sb[p0:p0 + 64, pair, q0:q0 + QW],
                            start=True,
                            stop=True,
                        )
                    pt = ptp.tile([128, 2 * QW], BF16, tag="pt", name="pt")
                    nc.scalar.activation(
                        pt[:], st[:], mybir.ActivationFunctionType.Exp
                    )
                    pstate[pi]["pts"][kt] = pt

                def OV(pi, kt):
                    qq, pair = PAIR_SEQ[pi]
                    ps = pstate[pi]
                    if kt == 0:
                        ps["ovA"] = ovp.tile([HEAD_DIM + 1, QW], F32,
                                             tag="ov", name="ovA")
                        ps["ovB"] = ovp.tile([HEAD_DIM + 1, QW], F32,
                                             tag="ov", name="ovB")
                    pt = ps["pts"].pop(kt)
                    for half, ov in ((0, ps["ovA"]), (1, ps["ovB"])):
                        nc.tensor.matmul(
                            ov[:],
                            v1_sb[:, kt, 2 * pair + half, :],
                            pt[:, half * QW:(half + 1) * QW],
                            start=(kt == 0),
                            stop=(kt == NT - 1),
                        )
                    if kt == NT - 1:
                        if pi == len(PAIR_SEQ) - 1:
                            normalize_tail(qq, pair, ps["ovA"], ps["ovB"])
                        else:
                            normalize(qq, pair, ps["ovA"], ps["ovB"])

                NSTEP = len(PAIR_SEQ) * NT
                LAG = 3
                for step in range(NSTEP + LAG):
                    if step < NSTEP:
                        pi, kt = step // NT, step % NT
                        if kt == 0:
                            pstate[pi] = {"pts": {}}
                        S(pi, kt)
                    if step >= LAG:
                        OV((step - LAG) // NT, (step - LAG) % NT)
                    for f in sched.get(step, ()):
                        f()

                # tail: last window's out-proj, PSUM tiles rotated through
                # the now-idle st/pp2 slots, writes split across both rings
                tail_slots = [(stp, "st"), (pp2s, "pp2"), (pps, "pp"),
                              (stp, "st")]
                i = 0
                for nt_i in range(QW // 128):
                    for cok in range(D_MODEL // QW):
                        pool, tag = tail_slots[i % len(tail_slots)]
                        # all tail writes on the sync HWDGE ring: the
                        # gpsimd/SWDGE queue has a ~4.5us Q7 drain that
                        # would land after the final transfer
                        y_unit(3, nt_i, cok, ps_pool=pool, ps_tag=tag,
                               ring=nc.sync)
                        i += 1

    nc.compile()
    return nc


def kernel(x, Wq, bq, Wk, bk, Wv, bv, Wo, bo):
    x = np.asarray(x, dtype=np.float32)
    Wq = np.asarray(Wq, dtype=np.float32)
    Wk = np.asarray(Wk, dtype=np.float32)
    Wv = np.asarray(Wv, dtype=np.float32)
    Wo = np.asarray(Wo, dtype=np.float32)
    bq = np.asarray(bq, dtype=np.float32)
    bk = np.asarray(bk, dtype=np.float32)
    bv = np.asarray(bv, dtype=np.float32)
    bo = np.asarray(bo, dtype=np.float32)

    if "nc" not in _CACHE:
        _CACHE["nc"] = build_nc()
    nc = _CACHE["nc"]

    bf16 = ml_dtypes.bfloat16

    def pack_w(w):  # [D, M] -> [128, (D/128)*M] partition-major
        d, m = w.shape
        return np.ascontiguousarray(
            w.reshape(d // 128, 128, m).transpose(1, 0, 2).reshape(128, -1))

    s = 2.0 / np.sqrt(8.0)  # fold bipolar *2 and score scale (1/8 split per side)
    in_maps = []
    for core in range(N_CORES):
        b = core // (N_CORES // B)
        g = core % (N_CORES // B)
        ch = slice(g * C_LOC, (g + 1) * C_LOC)
        wk_p = pack_w(np.ascontiguousarray((s * Wk[ch, :]).T))
        wq_p = pack_w(np.ascontiguousarray((s * Wq[ch, :]).T))
        wv_p = pack_w(np.ascontiguousarray(Wv[ch, :].T))
        wo_p = pack_w(np.ascontiguousarray(Wo[:, ch].T))
        bq_f = (2.0 * bq[ch] - 1.0) / np.sqrt(8.0)
        bk_f = (2.0 * bk[ch] - 1.0) / np.sqrt(8.0)
        # x n-chunk-major: [128, nch, dc, 512]
        xt = np.ascontiguousarray(x[b].T)  # [D, N]
        x_p = np.ascontiguousarray(
            xt.reshape(DC, 128, NW, QW).transpose(1, 2, 0, 3)
        ).reshape(128, NW, DC * QW)
        in_maps.append({
            "xP": x_p.astype(bf16),
            "wall": np.concatenate([wk_p, wq_p, wv_p, wo_p],
                                   axis=1).astype(bf16),
            "ball": np.concatenate(
                [bq_f.reshape(CT, 128).T, bk_f.reshape(CT, 128).T],
                axis=1).astype(np.float32),
        })

    _CACHE["in_maps"] = in_maps
    res = run_bass_kernel_spmd(nc, in_maps, core_ids=list(range(N_CORES)))

    g_per_b = N_CORES // B
    const = (Wo @ bv + bo).astype(np.float32)  # bv folded through out-proj
    out = np.empty((B, N, D_MODEL), dtype=np.float32)
    for b in range(B):
        acc = res.results[b * g_per_b]["y"].astype(np.float32).copy()
        for g in range(1, g_per_b):
            acc += res.results[b * g_per_b + g]["y"]
        out[b] = acc + const
    return out
